# revision 5
# baseline (speedup 1.0000x reference)
"""Trainium2 Bass kernel for single-layer LSTM (DPLSTM forward).

Problem: x [T=512, B=128, D=1024], w_ih [4096, 1024], w_hh [4096, 1024],
b_ih/b_hh [4096]. Returns (out [T,B,H], (h_n [B,H], c_n [B,H])).

Strategy (batch-parallel across 8 NeuronCores, per the sharding hint):
  - each core owns a 16-element batch slice; weights replicated.
  - transposed layout on-chip: gates^T [4096, 16] (gate rows on SBUF
    partitions, batch on the free dim) so all elementwise work runs on
    full 128-lane tiles and h^T feeds the next matmul without transposes.
  - fp16 matmuls (fp32 matmul is 4 cycles/row on TRN2; fp16 is 1),
    fp32 PSUM accumulation and fp32 cell state.
  - input projections gx = w_ih @ x^T + bias precomputed per 8-step
    chunk (amortizes w_ih weight loads over 128 moving columns).
  - one For_i loop over 64 chunks keeps the program ~2.5K instructions.

Host side shards/transposes inputs per core (numpy), runs the SPMD
kernel via run_bass_kernel_spmd on cores 0-7, and reassembles the
full [T, B, H] output (device writes [T, H, 16] per core; the final
transpose is host-side numpy).
"""

import contextlib
import ctypes
import sys
import types

import numpy as np

import bass_rust
import concourse.bass as bass
import concourse.mybir as mybir
import concourse.tile as tile
from concourse.bass_utils import run_bass_kernel_spmd

T, B, D, H = 512, 128, 1024, 1024
G4 = 4 * H  # 4096 gate rows
NCORES = 8
BL = B // NCORES  # 16 batch per core
CH = 8  # timesteps per chunk
NCH = T // CH
KD = D // 128  # 8 k-chunks over D
KH = H // 128  # 8 k-chunks over H
M = G4 // 128  # 32 gate partition-tiles

F16 = mybir.dt.float16
F32 = mybir.dt.float32
AF = mybir.ActivationFunctionType


# ---------------------------------------------------------------- infra
def _legalize_waits(nc):
    """This walrus accepts at most ONE sem-wait per instruction; hoist
    extras onto InstEventSemaphore instructions inserted just before."""
    cnt = 0
    for fn in nc.m.functions:
        for bb in fn.blocks:
            new_list = []
            changed = False
            for inst in bb.instructions:
                si = inst.sync_info
                if si is not None and len(si.on_wait) > 1:
                    waits = list(si.on_wait)
                    for w in waits[:-1]:
                        cnt += 1
                        ev = mybir.InstEventSemaphore(
                            name=f"I-lw-{cnt}", ins=[], outs=[]
                        )
                        ev.engine = inst.engine
                        ev.sync_info = bass_rust.SyncInfo(on_wait=[w], on_update=[])
                        new_list.append(ev)
                    inst.sync_info = bass_rust.SyncInfo(
                        on_wait=[waits[-1]], on_update=list(si.on_update)
                    )
                    changed = True
                new_list.append(inst)
            if changed:
                bb.instructions = new_list


def _install_ntff_hook(so_path="/opt/axon/libaxon_pjrt.so"):
    if "antenv.axon_hooks" in sys.modules:
        return
    try:
        lib = ctypes.CDLL(so_path)
    except OSError:
        return
    if not hasattr(lib, "axon_start_nrt_profile"):
        return
    lib.axon_start_nrt_profile.argtypes = [
        ctypes.POINTER(ctypes.c_int64),
        ctypes.c_size_t,
    ]
    lib.axon_start_nrt_profile.restype = ctypes.c_int64
    lib.axon_stop_nrt_profile.argtypes = [ctypes.c_char_p]
    lib.axon_stop_nrt_profile.restype = ctypes.c_int64

    @contextlib.contextmanager
    def _hook(output_dir, device_ids):
        import jax

        jax.devices()
        if device_ids:
            ids = (ctypes.c_int64 * len(device_ids))(*device_ids)
            rc = lib.axon_start_nrt_profile(ids, len(device_ids))
        else:
            rc = lib.axon_start_nrt_profile(None, 0)
        if rc != 0:
            raise RuntimeError(f"axon_start_nrt_profile rc={rc}")
        try:
            yield
        finally:
            n = lib.axon_stop_nrt_profile(str(output_dir).encode())
            print(f"profile: {n} ntff file(s) -> {output_dir}", file=sys.stderr)

    mod = types.ModuleType("antenv.axon_hooks")
    mod.get_axon_ntff_profile_hook = lambda: _hook
    mod.set_axon_ntff_profile_hook = lambda h: None
    sys.modules["antenv.axon_hooks"] = mod


# ---------------------------------------------------------------- kernel build
def build_nc(t_steps=T):
    nch = t_steps // CH
    nc = bass.Bass()
    xt_ext = nc.declare_dram_parameter("xt", [D, t_steps * BL], F16, isOutput=False)
    wih_ext = nc.declare_dram_parameter("wih", [D, G4], F16, isOutput=False)
    whh_ext = nc.declare_dram_parameter("whh", [H, G4], F16, isOutput=False)
    bias_ext = nc.declare_dram_parameter("bias", [128, M], F32, isOutput=False)
    out_ext = nc.declare_dram_parameter("out", [t_steps * H, BL], F32, isOutput=True)
    cn_ext = nc.declare_dram_parameter("cn", [H, BL], F32, isOutput=True)

    with tile.TileContext(nc) as tc:
        with (
            tc.tile_pool(name="wpool", bufs=1) as wpool,
            tc.tile_pool(name="state", bufs=1) as state,
            tc.tile_pool(name="xstage", bufs=2) as xstage_p,
            tc.tile_pool(name="gxring", bufs=2) as gxring_p,
            tc.tile_pool(name="hstage", bufs=2) as hstage_p,
            tc.tile_pool(name="tmp", bufs=4) as tmp_p,
            tc.tile_pool(name="gates", bufs=2) as gates_p,
            tc.tile_pool(name="recps", bufs=2, space="PSUM") as recps_p,
            tc.tile_pool(name="gxps", bufs=2, space="PSUM") as gxps_p,
        ):
            wih_sb = wpool.tile([128, KD * G4], F16, tag="wih")
            whh_sb = wpool.tile([128, KH * G4], F16, tag="whh")
            bias_sb = wpool.tile([128, M], F32, tag="bias")
            for k in range(KD):
                nc.sync.dma_start(
                    wih_sb[:, G4 * k : G4 * (k + 1)], wih_ext[128 * k : 128 * (k + 1), :]
                )
            for k in range(KH):
                nc.sync.dma_start(
                    whh_sb[:, G4 * k : G4 * (k + 1)], whh_ext[128 * k : 128 * (k + 1), :]
                )
            nc.sync.dma_start(bias_sb[:], bias_ext[:])

            h16 = state.tile([128, KH * BL], F16, tag="h16")  # h^T, chunk k at 16k
            c32 = state.tile([128, KH * BL], F32, tag="c32")  # c^T
            nc.vector.memset(h16[:], 0.0)
            nc.vector.memset(c32[:], 0.0)

            with tc.For_i(0, t_steps * BL, CH * BL) as xcol:
                # ---- stage x^T chunk [1024, CH*BL]
                xst = xstage_p.tile([128, KD * CH * BL], F16, tag="xst")
                cw = CH * BL  # 128 moving cols per chunk
                for k in range(KD):
                    nc.sync.dma_start(
                        xst[:, cw * k : cw * (k + 1)],
                        xt_ext[128 * k : 128 * (k + 1), bass.ds(xcol, cw)],
                    )
                # ---- gx = w_ih @ x^T + bias  (per m-tile), stored fp16
                gxc = gxring_p.tile([128, M * cw], F16, tag="gxc")
                for m in range(M):
                    gps = gxps_p.tile([128, cw], F32, tag="gps")
                    for k in range(KD):
                        nc.tensor.matmul(
                            gps[:],
                            wih_sb[:, G4 * k + 128 * m : G4 * k + 128 * (m + 1)],
                            xst[:, cw * k : cw * (k + 1)],
                            start=(k == 0),
                            stop=(k == KD - 1),
                        )
                    nc.vector.tensor_scalar_add(
                        gxc[:, cw * m : cw * (m + 1)], gps[:], bias_sb[:, m : m + 1]
                    )
                gxv = gxc.rearrange("p (m c) -> p m c", m=M)

                # ---- recurrence over the chunk's CH steps
                hst = hstage_p.tile([128, CH * KH * BL], F32, tag="hst")
                for tl in range(CH):
                    rps = recps_p.tile([128, M * BL], F32, tag="rps")
                    for m in range(M):
                        for k in range(KH):
                            nc.tensor.matmul(
                                rps[:, BL * m : BL * (m + 1)],
                                whh_sb[:, G4 * k + 128 * m : G4 * k + 128 * (m + 1)],
                                h16[:, BL * k : BL * (k + 1)],
                                start=(k == 0),
                                stop=(k == KH - 1),
                            )
                    # epilogue per gate (i, f, g, o), each 8 m-tiles = [128,128]
                    acts = []
                    for g4, func in enumerate([AF.Sigmoid, AF.Sigmoid, AF.Tanh, AF.Sigmoid]):
                        tmp = tmp_p.tile([128, KH * BL], F32, tag="pregate")
                        nc.vector.tensor_tensor(
                            tmp.rearrange("p (m c) -> p m c", m=KH),
                            rps.rearrange("p (m c) -> p m c", m=M)[
                                :, KH * g4 : KH * (g4 + 1), :
                            ],
                            gxv[:, KH * g4 : KH * (g4 + 1), bass.ds(BL * tl, BL)],
                            mybir.AluOpType.add,
                        )
                        gate = gates_p.tile([128, KH * BL], F32, tag=f"gate{g4}")
                        nc.scalar.activation(gate[:], tmp[:], func)
                        acts.append(gate)
                    sig_i, sig_f, tg, sig_o = acts
                    t1 = tmp_p.tile([128, KH * BL], F32, tag="t1")
                    nc.vector.tensor_tensor(t1[:], sig_i[:], tg[:], mybir.AluOpType.mult)
                    nc.vector.tensor_tensor(c32[:], sig_f[:], c32[:], mybir.AluOpType.mult)
                    nc.vector.tensor_tensor(c32[:], c32[:], t1[:], mybir.AluOpType.add)
                    th = tmp_p.tile([128, KH * BL], F32, tag="th")
                    nc.scalar.activation(th[:], c32[:], AF.Tanh)
                    hs = hst[:, KH * BL * tl : KH * BL * (tl + 1)]
                    nc.vector.tensor_tensor(hs, sig_o[:], th[:], mybir.AluOpType.mult)
                    nc.vector.tensor_copy(h16[:], hs)
                # ---- write chunk output: rows [8*1024] of out [t,h,b]
                trow = nc.snap(xcol * (H // BL))
                nc.sync.dma_start(
                    out_ext[bass.ds(trow, CH * H), :].rearrange(
                        "(t k p) b -> p t k b", t=CH, k=KH, p=128
                    ),
                    hst.rearrange("p (t k b) -> p t k b", t=CH, k=KH),
                )

            nc.sync.dma_start(
                cn_ext.rearrange("(k p) b -> p k b", p=128),
                c32.rearrange("p (k b) -> p k b", k=KH),
            )

    _legalize_waits(nc)
    return nc


_NC_CACHE = {}


def _get_nc(t_steps):
    if t_steps not in _NC_CACHE:
        _NC_CACHE[t_steps] = build_nc(t_steps)
    return _NC_CACHE[t_steps]


# ---------------------------------------------------------------- host wrapper
def kernel(x, w_ih, b_ih, w_hh, b_hh, trace=False):
    x = np.asarray(x)
    w_ih = np.asarray(w_ih)
    b_ih = np.asarray(b_ih)
    w_hh = np.asarray(w_hh)
    b_hh = np.asarray(b_hh)
    t_steps = x.shape[0]

    _install_ntff_hook()
    nc = _get_nc(t_steps)

    wihT = np.ascontiguousarray(w_ih.T).astype(np.float16)  # [D, 4H]
    whhT = np.ascontiguousarray(w_hh.T).astype(np.float16)  # [H, 4H]
    bias = (b_ih + b_hh).astype(np.float32).reshape(M, 128).T.copy()  # [128, M]

    in_maps = []
    for c in range(NCORES):
        sl = slice(BL * c, BL * (c + 1))
        # x^T slice: [D, T, BL] -> [D, T*BL] fp16
        xt = np.ascontiguousarray(x[:, sl, :].transpose(2, 0, 1)).reshape(
            D, t_steps * BL
        ).astype(np.float16)
        in_maps.append(
            {"xt": xt, "wih": wihT.copy(), "whh": whhT.copy(), "bias": bias.copy()}
        )

    res = run_bass_kernel_spmd(nc, in_maps, list(range(NCORES)), trace=trace)

    out = np.empty((t_steps, B, H), np.float32)
    c_n = np.empty((B, H), np.float32)
    for c in range(NCORES):
        sl = slice(BL * c, BL * (c + 1))
        oc = res.results[c]["out"].reshape(t_steps, H, BL)
        out[:, sl, :] = oc.transpose(0, 2, 1)
        c_n[sl, :] = res.results[c]["cn"].T
    h_n = out[-1].copy()
    if trace:
        kernel.last_exec_ns = res.exec_time_ns
    return out, (h_n, c_n)


# revision 7
# speedup vs baseline: 1.1684x; 1.1684x over previous
"""Trainium2 Bass kernel for single-layer LSTM (DPLSTM forward).

Problem: x [T=512, B=128, D=1024], w_ih [4096, 1024], w_hh [4096, 1024],
b_ih/b_hh [4096]. Returns (out [T,B,H], (h_n [B,H], c_n [B,H])).

Strategy (batch-parallel across 8 NeuronCores, per the sharding hint):
  - each core owns a 16-element batch slice; weights replicated.
  - transposed layout on-chip: gates^T [4096, 16] (gate rows on SBUF
    partitions, batch on the free dim) so all elementwise work runs on
    full 128-lane tiles and h^T feeds the next matmul without transposes.
  - fp16 matmuls (fp32 matmul is 4 cycles/row on TRN2; fp16 is 1),
    fp32 PSUM accumulation and fp32 cell state.
  - input projections gx = w_ih @ x^T + bias are computed one 8-step
    chunk AHEAD of the recurrence (software pipeline with two buffers),
    so the gx matmuls fill the PE stalls left by the per-step serial
    epilogue (activations + cell update).
  - per-gate PSUM tiles (4 banks) let gate i's epilogue overlap gate
    f/g/o matmuls.
  - one For_i loop over 32 chunk-pairs keeps the program small.

Host side shards/transposes inputs per core (numpy), runs the SPMD
kernel via run_bass_kernel_spmd on cores 0-7, and reassembles the full
[T, B, H] output (device writes [T, H, 16] per core; final transpose is
host-side numpy).
"""

import contextlib
import ctypes
import sys
import types

import numpy as np

import bass_rust
import concourse.bass as bass
import concourse.mybir as mybir
import concourse.tile as tile
from concourse.bass_utils import run_bass_kernel_spmd

T, B, D, H = 512, 128, 1024, 1024
G4 = 4 * H  # 4096 gate rows
NCORES = 8
BL = B // NCORES  # 16 batch per core
CH = 8  # timesteps per chunk
KD = D // 128  # 8 k-chunks over D
KH = H // 128  # 8 k-chunks over H
M = G4 // 128  # 32 gate partition-tiles
CW = CH * BL  # moving columns per chunk (128)

F16 = mybir.dt.float16
F32 = mybir.dt.float32
AF = mybir.ActivationFunctionType


# ---------------------------------------------------------------- infra
def _legalize_waits(nc):
    """This walrus accepts at most ONE sem-wait per instruction; hoist
    extras onto InstEventSemaphore instructions inserted just before."""
    cnt = 0
    for fn in nc.m.functions:
        for bb in fn.blocks:
            new_list = []
            changed = False
            for inst in bb.instructions:
                si = inst.sync_info
                if si is not None and len(si.on_wait) > 1:
                    waits = list(si.on_wait)
                    for w in waits[:-1]:
                        cnt += 1
                        ev = mybir.InstEventSemaphore(
                            name=f"I-lw-{cnt}", ins=[], outs=[]
                        )
                        ev.engine = inst.engine
                        ev.sync_info = bass_rust.SyncInfo(on_wait=[w], on_update=[])
                        new_list.append(ev)
                    inst.sync_info = bass_rust.SyncInfo(
                        on_wait=[waits[-1]], on_update=list(si.on_update)
                    )
                    changed = True
                new_list.append(inst)
            if changed:
                bb.instructions = new_list


def _install_ntff_hook(so_path="/opt/axon/libaxon_pjrt.so"):
    if "antenv.axon_hooks" in sys.modules:
        return
    try:
        lib = ctypes.CDLL(so_path)
    except OSError:
        return
    if not hasattr(lib, "axon_start_nrt_profile"):
        return
    lib.axon_start_nrt_profile.argtypes = [
        ctypes.POINTER(ctypes.c_int64),
        ctypes.c_size_t,
    ]
    lib.axon_start_nrt_profile.restype = ctypes.c_int64
    lib.axon_stop_nrt_profile.argtypes = [ctypes.c_char_p]
    lib.axon_stop_nrt_profile.restype = ctypes.c_int64

    @contextlib.contextmanager
    def _hook(output_dir, device_ids):
        import jax

        jax.devices()
        if device_ids:
            ids = (ctypes.c_int64 * len(device_ids))(*device_ids)
            rc = lib.axon_start_nrt_profile(ids, len(device_ids))
        else:
            rc = lib.axon_start_nrt_profile(None, 0)
        if rc != 0:
            raise RuntimeError(f"axon_start_nrt_profile rc={rc}")
        try:
            yield
        finally:
            n = lib.axon_stop_nrt_profile(str(output_dir).encode())
            print(f"profile: {n} ntff file(s) -> {output_dir}", file=sys.stderr)

    mod = types.ModuleType("antenv.axon_hooks")
    mod.get_axon_ntff_profile_hook = lambda: _hook
    mod.set_axon_ntff_profile_hook = lambda h: None
    sys.modules["antenv.axon_hooks"] = mod


# ---------------------------------------------------------------- kernel build
def build_nc(t_steps=T):
    nc = bass.Bass()
    # xt is padded by one chunk so the pipelined gx prefetch can run one
    # chunk ahead without a guard.
    xt_ext = nc.declare_dram_parameter(
        "xt", [D, (t_steps + CH) * BL], F16, isOutput=False
    )
    wih_ext = nc.declare_dram_parameter("wih", [D, G4], F16, isOutput=False)
    whh_ext = nc.declare_dram_parameter("whh", [H, G4], F16, isOutput=False)
    bias_ext = nc.declare_dram_parameter("bias", [128, M], F32, isOutput=False)
    out_ext = nc.declare_dram_parameter("out", [t_steps * H, BL], F32, isOutput=True)
    cn_ext = nc.declare_dram_parameter("cn", [H, BL], F32, isOutput=True)

    with tile.TileContext(nc) as tc:
        with (
            tc.tile_pool(name="wpool", bufs=1) as wpool,
            tc.tile_pool(name="state", bufs=1) as state,
            tc.tile_pool(name="xstage", bufs=3) as xstage_p,
            tc.tile_pool(name="hstage", bufs=2) as hstage_p,
            tc.tile_pool(name="tmp", bufs=6) as tmp_p,
            tc.tile_pool(name="gates", bufs=2) as gates_p,
            tc.tile_pool(name="recps", bufs=1, space="PSUM") as recps_p,
            tc.tile_pool(name="gxps", bufs=2, space="PSUM") as gxps_p,
        ):
            wih_sb = wpool.tile([128, KD * G4], F16, tag="wih")
            whh_sb = wpool.tile([128, KH * G4], F16, tag="whh")
            bias_sb = wpool.tile([128, M], F32, tag="bias")
            for k in range(KD):
                nc.sync.dma_start(
                    wih_sb[:, G4 * k : G4 * (k + 1)],
                    wih_ext[128 * k : 128 * (k + 1), :],
                )
            for k in range(KH):
                nc.sync.dma_start(
                    whh_sb[:, G4 * k : G4 * (k + 1)],
                    whh_ext[128 * k : 128 * (k + 1), :],
                )
            nc.sync.dma_start(bias_sb[:], bias_ext[:])

            h16 = state.tile([128, KH * BL], F16, tag="h16")  # h^T, chunk k at 16k
            c32 = state.tile([128, KH * BL], F32, tag="c32")  # c^T
            gx_a = state.tile([128, M * CW], F16, tag="gx_a")
            gx_b = state.tile([128, M * CW], F16, tag="gx_b")
            nc.vector.memset(h16[:], 0.0)
            nc.vector.memset(c32[:], 0.0)

            def gx_phase(xcol, gxc):
                """gxc[:, m*CW + t*BL + b] = (w_ih @ x^T + bias) for one chunk."""
                xst = xstage_p.tile([128, KD * CW], F16, tag="xst")
                for k in range(KD):
                    nc.sync.dma_start(
                        xst[:, CW * k : CW * (k + 1)],
                        xt_ext[128 * k : 128 * (k + 1), bass.ds(xcol, CW)],
                    )
                for m in range(M):
                    gps = gxps_p.tile([128, CW], F32, tag="gps")
                    for k in range(KD):
                        nc.tensor.matmul(
                            gps[:],
                            wih_sb[:, G4 * k + 128 * m : G4 * k + 128 * (m + 1)],
                            xst[:, CW * k : CW * (k + 1)],
                            start=(k == 0),
                            stop=(k == KD - 1),
                        )
                    nc.vector.tensor_scalar_add(
                        gxc[:, CW * m : CW * (m + 1)], gps[:], bias_sb[:, m : m + 1]
                    )

            def rec_phase(trow, gxc):
                """Run CH recurrence steps consuming gxc; write h chunk out."""
                gxv = gxc.rearrange("p (m c) -> p m c", m=M)
                hst = hstage_p.tile([128, CH * KH * BL], F32, tag="hst")
                for tl in range(CH):
                    rps = [
                        recps_p.tile([128, KH * BL], F32, tag=f"rps{g}", name=f"rps{g}")
                        for g in range(4)
                    ]
                    for m in range(M):
                        g4, mj = divmod(m, KH)
                        for k in range(KH):
                            nc.tensor.matmul(
                                rps[g4][:, BL * mj : BL * (mj + 1)],
                                whh_sb[:, G4 * k + 128 * m : G4 * k + 128 * (m + 1)],
                                h16[:, BL * k : BL * (k + 1)],
                                start=(k == 0),
                                stop=(k == KH - 1),
                            )
                    acts = []
                    for g4, func in enumerate(
                        [AF.Sigmoid, AF.Sigmoid, AF.Tanh, AF.Sigmoid]
                    ):
                        tmp = tmp_p.tile([128, KH * BL], F32, tag="pregate")
                        nc.vector.tensor_tensor(
                            tmp.rearrange("p (m c) -> p m c", m=KH),
                            rps[g4].rearrange("p (m c) -> p m c", m=KH),
                            gxv[:, KH * g4 : KH * (g4 + 1), bass.ds(BL * tl, BL)],
                            mybir.AluOpType.add,
                        )
                        gate = gates_p.tile([128, KH * BL], F32, tag=f"gate{g4}")
                        nc.scalar.activation(gate[:], tmp[:], func)
                        acts.append(gate)
                    sig_i, sig_f, tg, sig_o = acts
                    t1 = tmp_p.tile([128, KH * BL], F32, tag="t1")
                    nc.vector.tensor_tensor(
                        t1[:], sig_i[:], tg[:], mybir.AluOpType.mult
                    )
                    nc.vector.tensor_tensor(
                        c32[:], sig_f[:], c32[:], mybir.AluOpType.mult
                    )
                    nc.vector.tensor_tensor(
                        c32[:], c32[:], t1[:], mybir.AluOpType.add
                    )
                    th = tmp_p.tile([128, KH * BL], F32, tag="th")
                    nc.scalar.activation(th[:], c32[:], AF.Tanh)
                    hs = hst[:, KH * BL * tl : KH * BL * (tl + 1)]
                    nc.vector.tensor_tensor(
                        hs, sig_o[:], th[:], mybir.AluOpType.mult
                    )
                    nc.vector.tensor_copy(h16[:], hs)
                nc.sync.dma_start(
                    out_ext[bass.ds(trow, CH * H), :].rearrange(
                        "(t k p) b -> p t k b", t=CH, k=KH, p=128
                    ),
                    hst.rearrange("p (t k b) -> p t k b", t=CH, k=KH),
                )

            # prologue: gx for chunk 0
            gx_phase(0, gx_a)
            # chunk pairs: rec(2j) from A while gx(2j+1)->B, then
            # rec(2j+1) from B while gx(2j+2)->A.
            with tc.For_i(0, t_steps * BL, 2 * CW) as xcol:
                gx_phase(nc.snap(xcol + CW), gx_b)
                rec_phase(nc.snap(xcol * (H // BL)), gx_a)
                gx_phase(nc.snap(xcol + 2 * CW), gx_a)
                rec_phase(nc.snap(xcol * (H // BL) + CH * H), gx_b)

            nc.sync.dma_start(
                cn_ext.rearrange("(k p) b -> p k b", p=128),
                c32.rearrange("p (k b) -> p k b", k=KH),
            )

    _legalize_waits(nc)
    return nc


_NC_CACHE = {}


def _get_nc(t_steps):
    if t_steps not in _NC_CACHE:
        _NC_CACHE[t_steps] = build_nc(t_steps)
    return _NC_CACHE[t_steps]


# ---------------------------------------------------------------- host wrapper
def kernel(x, w_ih, b_ih, w_hh, b_hh, trace=False):
    x = np.asarray(x)
    w_ih = np.asarray(w_ih)
    b_ih = np.asarray(b_ih)
    w_hh = np.asarray(w_hh)
    b_hh = np.asarray(b_hh)
    t_steps = x.shape[0]

    _install_ntff_hook()
    nc = _get_nc(t_steps)

    wihT = np.ascontiguousarray(w_ih.T).astype(np.float16)  # [D, 4H]
    whhT = np.ascontiguousarray(w_hh.T).astype(np.float16)  # [H, 4H]
    bias = (b_ih + b_hh).astype(np.float32).reshape(M, 128).T.copy()  # [128, M]

    in_maps = []
    for c in range(NCORES):
        sl = slice(BL * c, BL * (c + 1))
        # x^T slice: [D, T, BL] -> [D, (T+CH)*BL] fp16, padded one chunk
        xt = np.zeros((D, (t_steps + CH) * BL), np.float16)
        xt[:, : t_steps * BL] = (
            np.ascontiguousarray(x[:, sl, :].transpose(2, 0, 1))
            .reshape(D, t_steps * BL)
            .astype(np.float16)
        )
        in_maps.append(
            {"xt": xt, "wih": wihT.copy(), "whh": whhT.copy(), "bias": bias.copy()}
        )

    res = run_bass_kernel_spmd(nc, in_maps, list(range(NCORES)), trace=trace)

    out = np.empty((t_steps, B, H), np.float32)
    c_n = np.empty((B, H), np.float32)
    for c in range(NCORES):
        sl = slice(BL * c, BL * (c + 1))
        oc = res.results[c]["out"].reshape(t_steps, H, BL)
        out[:, sl, :] = oc.transpose(0, 2, 1)
        c_n[sl, :] = res.results[c]["cn"].T
    h_n = out[-1].copy()
    if trace:
        kernel.last_exec_ns = res.exec_time_ns
    return out, (h_n, c_n)


# revision 12
# speedup vs baseline: 1.1935x; 1.0215x over previous
"""Trainium2 Bass kernel for single-layer LSTM (DPLSTM forward).

Problem: x [T=512, B=128, D=1024], w_ih [4096, 1024], w_hh [4096, 1024],
b_ih/b_hh [4096]. Returns (out [T,B,H], (h_n [B,H], c_n [B,H])).

Strategy (batch-parallel across 8 NeuronCores, per the sharding hint):
  - each core owns a 16-element batch slice; weights replicated.
  - transposed layout on-chip: gates^T [4096, 16] (gate rows on SBUF
    partitions, batch on the free dim) so all elementwise work runs on
    full 128-lane tiles and h^T feeds the next matmul without transposes.
  - fp16 matmuls (fp32 matmul is 4 cycles/row on TRN2; fp16 is 1),
    fp32 PSUM accumulation and fp32 cell state.
  - input projections gx = w_ih @ x^T + bias are computed one 8-step
    chunk AHEAD of the recurrence (software pipeline with two buffers),
    so the gx matmuls fill the PE stalls left by the per-step serial
    epilogue (activations + cell update).
  - per-gate PSUM tiles (4 banks) let gate i's epilogue overlap gate
    f/g/o matmuls.
  - one For_i loop over 32 chunk-pairs keeps the program small.

Host side shards/transposes inputs per core (numpy), runs the SPMD
kernel via run_bass_kernel_spmd on cores 0-7, and reassembles the full
[T, B, H] output (device writes [T, H, 16] per core; final transpose is
host-side numpy).
"""

import contextlib
import ctypes
import sys
import types

import numpy as np

import bass_rust
import concourse.bass as bass
import concourse.mybir as mybir
import concourse.tile as tile
from concourse.bass_utils import run_bass_kernel_spmd

T, B, D, H = 512, 128, 1024, 1024
G4 = 4 * H  # 4096 gate rows
NCORES = 8
BL = B // NCORES  # 16 batch per core
CH = 8  # timesteps per chunk
KD = D // 128  # 8 k-chunks over D
KH = H // 128  # 8 k-chunks over H
M = G4 // 128  # 32 gate partition-tiles
CW = CH * BL  # moving columns per chunk (128)

F16 = mybir.dt.float16
F32 = mybir.dt.float32
AF = mybir.ActivationFunctionType


# ---------------------------------------------------------------- infra
def _legalize_waits(nc):
    """This walrus accepts at most ONE sem-wait per instruction; hoist
    extras onto InstEventSemaphore instructions inserted just before."""
    cnt = 0
    for fn in nc.m.functions:
        for bb in fn.blocks:
            new_list = []
            changed = False
            for inst in bb.instructions:
                si = inst.sync_info
                if si is not None and len(si.on_wait) > 1:
                    waits = list(si.on_wait)
                    for w in waits[:-1]:
                        cnt += 1
                        ev = mybir.InstEventSemaphore(
                            name=f"I-lw-{cnt}", ins=[], outs=[]
                        )
                        ev.engine = inst.engine
                        ev.sync_info = bass_rust.SyncInfo(on_wait=[w], on_update=[])
                        new_list.append(ev)
                    inst.sync_info = bass_rust.SyncInfo(
                        on_wait=[waits[-1]], on_update=list(si.on_update)
                    )
                    changed = True
                new_list.append(inst)
            if changed:
                bb.instructions = new_list


def _install_ntff_hook(so_path="/opt/axon/libaxon_pjrt.so"):
    if "antenv.axon_hooks" in sys.modules:
        return
    try:
        lib = ctypes.CDLL(so_path)
    except OSError:
        return
    if not hasattr(lib, "axon_start_nrt_profile"):
        return
    lib.axon_start_nrt_profile.argtypes = [
        ctypes.POINTER(ctypes.c_int64),
        ctypes.c_size_t,
    ]
    lib.axon_start_nrt_profile.restype = ctypes.c_int64
    lib.axon_stop_nrt_profile.argtypes = [ctypes.c_char_p]
    lib.axon_stop_nrt_profile.restype = ctypes.c_int64

    @contextlib.contextmanager
    def _hook(output_dir, device_ids):
        import jax

        jax.devices()
        if device_ids:
            ids = (ctypes.c_int64 * len(device_ids))(*device_ids)
            rc = lib.axon_start_nrt_profile(ids, len(device_ids))
        else:
            rc = lib.axon_start_nrt_profile(None, 0)
        if rc != 0:
            raise RuntimeError(f"axon_start_nrt_profile rc={rc}")
        try:
            yield
        finally:
            n = lib.axon_stop_nrt_profile(str(output_dir).encode())
            print(f"profile: {n} ntff file(s) -> {output_dir}", file=sys.stderr)

    mod = types.ModuleType("antenv.axon_hooks")
    mod.get_axon_ntff_profile_hook = lambda: _hook
    mod.set_axon_ntff_profile_hook = lambda h: None
    sys.modules["antenv.axon_hooks"] = mod


# ---------------------------------------------------------------- kernel build
def build_nc(t_steps=T):
    nc = bass.Bass()
    # xt is padded by one chunk so the pipelined gx prefetch can run one
    # chunk ahead without a guard.
    xt_ext = nc.declare_dram_parameter(
        "xt", [D, (t_steps + CH) * BL], F16, isOutput=False
    )
    wih_ext = nc.declare_dram_parameter("wih", [D, G4], F16, isOutput=False)
    whh_ext = nc.declare_dram_parameter("whh", [H, G4], F16, isOutput=False)
    bias_ext = nc.declare_dram_parameter("bias", [128, M], F32, isOutput=False)
    out_ext = nc.declare_dram_parameter("out", [t_steps * H, BL], F16, isOutput=True)
    cn_ext = nc.declare_dram_parameter("cn", [H, BL], F32, isOutput=True)

    with tile.TileContext(nc) as tc:
        with (
            tc.tile_pool(name="wpool", bufs=1) as wpool,
            tc.tile_pool(name="state", bufs=1) as state,
            tc.tile_pool(name="xstage", bufs=3) as xstage_p,
            tc.tile_pool(name="hstage", bufs=2) as hstage_p,
            tc.tile_pool(name="tmp", bufs=6) as tmp_p,
            tc.tile_pool(name="gates", bufs=2) as gates_p,
            tc.tile_pool(name="recps", bufs=1, space="PSUM") as recps_p,
            tc.tile_pool(name="gxps", bufs=2, space="PSUM") as gxps_p,
        ):
            wih_sb = wpool.tile([128, KD * G4], F16, tag="wih")
            whh_sb = wpool.tile([128, KH * G4], F16, tag="whh")
            bias_sb = wpool.tile([128, M], F32, tag="bias")
            for k in range(KD):
                nc.sync.dma_start(
                    wih_sb[:, G4 * k : G4 * (k + 1)],
                    wih_ext[128 * k : 128 * (k + 1), :],
                )
            for k in range(KH):
                nc.sync.dma_start(
                    whh_sb[:, G4 * k : G4 * (k + 1)],
                    whh_ext[128 * k : 128 * (k + 1), :],
                )
            nc.sync.dma_start(bias_sb[:], bias_ext[:])

            h16 = state.tile([128, KH * BL], F16, tag="h16")  # h^T, chunk k at 16k
            c32 = state.tile([128, KH * BL], F32, tag="c32")  # c^T
            gx_a = state.tile([128, M * CW], F16, tag="gx_a")
            gx_b = state.tile([128, M * CW], F16, tag="gx_b")
            nc.vector.memset(h16[:], 0.0)
            nc.vector.memset(c32[:], 0.0)

            def gx_phase(xcol, gxc):
                """gxc[:, m*CW + t*BL + b] = (w_ih @ x^T + bias) for one chunk."""
                xst = xstage_p.tile([128, KD * CW], F16, tag="xst")
                for k in range(KD):
                    nc.sync.dma_start(
                        xst[:, CW * k : CW * (k + 1)],
                        xt_ext[128 * k : 128 * (k + 1), bass.ds(xcol, CW)],
                    )
                for m in range(M):
                    gps = gxps_p.tile([128, CW], F32, tag="gps")
                    for k in range(KD):
                        nc.tensor.matmul(
                            gps[:],
                            wih_sb[:, G4 * k + 128 * m : G4 * k + 128 * (m + 1)],
                            xst[:, CW * k : CW * (k + 1)],
                            start=(k == 0),
                            stop=(k == KD - 1),
                        )
                    nc.vector.tensor_scalar_add(
                        gxc[:, CW * m : CW * (m + 1)], gps[:], bias_sb[:, m : m + 1]
                    )

            def rec_phase(trow, gxc):
                """Run CH recurrence steps consuming gxc; write h chunk out.

                h lives in fp16 directly (hst slices double as the next
                step's matmul operand); h16 carries state across chunks.
                """
                gxv = gxc.rearrange("p (m c) -> p m c", m=M)
                hst = hstage_p.tile([128, CH * KH * BL], F16, tag="hst")
                h_cur = h16[:]
                for tl in range(CH):
                    rps = [
                        recps_p.tile([128, KH * BL], F32, tag=f"rps{g}", name=f"rps{g}")
                        for g in range(4)
                    ]
                    for m in range(M):
                        g4, mj = divmod(m, KH)
                        for k in range(KH):
                            nc.tensor.matmul(
                                rps[g4][:, BL * mj : BL * (mj + 1)],
                                whh_sb[:, G4 * k + 128 * m : G4 * k + 128 * (m + 1)],
                                h_cur[:, BL * k : BL * (k + 1)],
                                start=(k == 0),
                                stop=(k == KH - 1),
                            )
                    acts = []
                    for g4, func in enumerate(
                        [AF.Sigmoid, AF.Sigmoid, AF.Tanh, AF.Sigmoid]
                    ):
                        tmp = tmp_p.tile([128, KH * BL], F32, tag="pregate")
                        nc.vector.tensor_tensor(
                            tmp.rearrange("p (m c) -> p m c", m=KH),
                            rps[g4].rearrange("p (m c) -> p m c", m=KH),
                            gxv[:, KH * g4 : KH * (g4 + 1), bass.ds(BL * tl, BL)],
                            mybir.AluOpType.add,
                        )
                        gate = gates_p.tile([128, KH * BL], F32, tag=f"gate{g4}")
                        nc.scalar.activation(gate[:], tmp[:], func)
                        acts.append(gate)
                    sig_i, sig_f, tg, sig_o = acts
                    t1 = tmp_p.tile([128, KH * BL], F32, tag="t1")
                    nc.vector.tensor_tensor(
                        t1[:], sig_i[:], tg[:], mybir.AluOpType.mult
                    )
                    nc.vector.tensor_tensor(
                        c32[:], sig_f[:], c32[:], mybir.AluOpType.mult
                    )
                    nc.vector.tensor_tensor(
                        c32[:], c32[:], t1[:], mybir.AluOpType.add
                    )
                    th = tmp_p.tile([128, KH * BL], F32, tag="th")
                    nc.scalar.activation(th[:], c32[:], AF.Tanh)
                    hs = hst[:, KH * BL * tl : KH * BL * (tl + 1)]
                    nc.vector.tensor_tensor(
                        hs, sig_o[:], th[:], mybir.AluOpType.mult
                    )
                    h_cur = hs
                nc.vector.tensor_copy(h16[:], h_cur)
                nc.sync.dma_start(
                    out_ext[bass.ds(trow, CH * H), :].rearrange(
                        "(t k p) b -> p t k b", t=CH, k=KH, p=128
                    ),
                    hst.rearrange("p (t k b) -> p t k b", t=CH, k=KH),
                )

            # prologue: gx for chunk 0
            gx_phase(0, gx_a)
            # chunk pairs: rec(2j) from A while gx(2j+1)->B, then
            # rec(2j+1) from B while gx(2j+2)->A.
            with tc.For_i(0, t_steps * BL, 2 * CW) as xcol:
                gx_phase(nc.snap(xcol + CW), gx_b)
                rec_phase(nc.snap(xcol * (H // BL)), gx_a)
                gx_phase(nc.snap(xcol + 2 * CW), gx_a)
                rec_phase(nc.snap(xcol * (H // BL) + CH * H), gx_b)

            nc.sync.dma_start(
                cn_ext.rearrange("(k p) b -> p k b", p=128),
                c32.rearrange("p (k b) -> p k b", k=KH),
            )

    _legalize_waits(nc)
    return nc


_NC_CACHE = {}


def _get_nc(t_steps):
    if t_steps not in _NC_CACHE:
        _NC_CACHE[t_steps] = build_nc(t_steps)
    return _NC_CACHE[t_steps]


# ---------------------------------------------------------------- host wrapper
def kernel(x, w_ih, b_ih, w_hh, b_hh, trace=False):
    x = np.asarray(x)
    w_ih = np.asarray(w_ih)
    b_ih = np.asarray(b_ih)
    w_hh = np.asarray(w_hh)
    b_hh = np.asarray(b_hh)
    t_steps = x.shape[0]

    _install_ntff_hook()
    nc = _get_nc(t_steps)

    wihT = np.ascontiguousarray(w_ih.T).astype(np.float16)  # [D, 4H]
    whhT = np.ascontiguousarray(w_hh.T).astype(np.float16)  # [H, 4H]
    bias = (b_ih + b_hh).astype(np.float32).reshape(M, 128).T.copy()  # [128, M]

    in_maps = []
    for c in range(NCORES):
        sl = slice(BL * c, BL * (c + 1))
        # x^T slice: [D, T, BL] -> [D, (T+CH)*BL] fp16, padded one chunk
        xt = np.zeros((D, (t_steps + CH) * BL), np.float16)
        xt[:, : t_steps * BL] = (
            np.ascontiguousarray(x[:, sl, :].transpose(2, 0, 1))
            .reshape(D, t_steps * BL)
            .astype(np.float16)
        )
        in_maps.append(
            {"xt": xt, "wih": wihT.copy(), "whh": whhT.copy(), "bias": bias.copy()}
        )

    res = run_bass_kernel_spmd(nc, in_maps, list(range(NCORES)), trace=trace)

    out = np.empty((t_steps, B, H), np.float32)
    c_n = np.empty((B, H), np.float32)
    for c in range(NCORES):
        sl = slice(BL * c, BL * (c + 1))
        oc = res.results[c]["out"].reshape(t_steps, H, BL).astype(np.float32)
        out[:, sl, :] = oc.transpose(0, 2, 1)
        c_n[sl, :] = res.results[c]["cn"].T
    h_n = out[-1].copy()
    if trace:
        kernel.last_exec_ns = res.exec_time_ns
    return out, (h_n, c_n)


# revision 15
# speedup vs baseline: 1.2309x; 1.0313x over previous
"""Trainium2 Bass kernel for single-layer LSTM (DPLSTM forward).

Problem: x [T=512, B=128, D=1024], w_ih [4096, 1024], w_hh [4096, 1024],
b_ih/b_hh [4096]. Returns (out [T,B,H], (h_n [B,H], c_n [B,H])).

Strategy (batch-parallel across 8 NeuronCores, per the sharding hint):
  - each core owns a 16-element batch slice; weights replicated.
  - transposed layout on-chip: gates^T [4096, 16] (gate rows on SBUF
    partitions, batch on the free dim) so all elementwise work runs on
    full 128-lane tiles and h^T feeds the next matmul without transposes.
  - fp16 matmuls (fp32 matmul is 4 cycles/row on TRN2; fp16 is 1),
    fp32 PSUM accumulation and fp32 cell state.
  - input projections gx = w_ih @ x^T + bias are computed one 8-step
    chunk AHEAD of the recurrence (software pipeline with two buffers),
    so the gx matmuls fill the PE stalls left by the per-step serial
    epilogue (activations + cell update).
  - per-gate PSUM tiles (4 banks) let gate i's epilogue overlap gate
    f/g/o matmuls.
  - one For_i loop over 32 chunk-pairs keeps the program small.

Host side shards/transposes inputs per core (numpy), runs the SPMD
kernel via run_bass_kernel_spmd on cores 0-7, and reassembles the full
[T, B, H] output (device writes [T, H, 16] per core; final transpose is
host-side numpy).
"""

import contextlib
import ctypes
import sys
import types

import numpy as np

import bass_rust
import concourse.bass as bass
import concourse.mybir as mybir
import concourse.tile as tile
from concourse.bass_utils import run_bass_kernel_spmd

T, B, D, H = 512, 128, 1024, 1024
G4 = 4 * H  # 4096 gate rows
NCORES = 8
BL = B // NCORES  # 16 batch per core
CH = 8  # timesteps per chunk
KD = D // 128  # 8 k-chunks over D
KH = H // 128  # 8 k-chunks over H
M = G4 // 128  # 32 gate partition-tiles
CW = CH * BL  # moving columns per chunk (128)

F16 = mybir.dt.float16
F32 = mybir.dt.float32
AF = mybir.ActivationFunctionType


# ---------------------------------------------------------------- infra
def _legalize_waits(nc):
    """This walrus accepts at most ONE sem-wait per instruction; hoist
    extras onto InstEventSemaphore instructions inserted just before."""
    cnt = 0
    for fn in nc.m.functions:
        for bb in fn.blocks:
            new_list = []
            changed = False
            for inst in bb.instructions:
                si = inst.sync_info
                if si is not None and len(si.on_wait) > 1:
                    waits = list(si.on_wait)
                    for w in waits[:-1]:
                        cnt += 1
                        ev = mybir.InstEventSemaphore(
                            name=f"I-lw-{cnt}", ins=[], outs=[]
                        )
                        ev.engine = inst.engine
                        ev.sync_info = bass_rust.SyncInfo(on_wait=[w], on_update=[])
                        new_list.append(ev)
                    inst.sync_info = bass_rust.SyncInfo(
                        on_wait=[waits[-1]], on_update=list(si.on_update)
                    )
                    changed = True
                new_list.append(inst)
            if changed:
                bb.instructions = new_list


def _install_ntff_hook(so_path="/opt/axon/libaxon_pjrt.so"):
    if "antenv.axon_hooks" in sys.modules:
        return
    try:
        lib = ctypes.CDLL(so_path)
    except OSError:
        return
    if not hasattr(lib, "axon_start_nrt_profile"):
        return
    lib.axon_start_nrt_profile.argtypes = [
        ctypes.POINTER(ctypes.c_int64),
        ctypes.c_size_t,
    ]
    lib.axon_start_nrt_profile.restype = ctypes.c_int64
    lib.axon_stop_nrt_profile.argtypes = [ctypes.c_char_p]
    lib.axon_stop_nrt_profile.restype = ctypes.c_int64

    @contextlib.contextmanager
    def _hook(output_dir, device_ids):
        import jax

        jax.devices()
        if device_ids:
            ids = (ctypes.c_int64 * len(device_ids))(*device_ids)
            rc = lib.axon_start_nrt_profile(ids, len(device_ids))
        else:
            rc = lib.axon_start_nrt_profile(None, 0)
        if rc != 0:
            raise RuntimeError(f"axon_start_nrt_profile rc={rc}")
        try:
            yield
        finally:
            n = lib.axon_stop_nrt_profile(str(output_dir).encode())
            print(f"profile: {n} ntff file(s) -> {output_dir}", file=sys.stderr)

    mod = types.ModuleType("antenv.axon_hooks")
    mod.get_axon_ntff_profile_hook = lambda: _hook
    mod.set_axon_ntff_profile_hook = lambda h: None
    sys.modules["antenv.axon_hooks"] = mod


# ---------------------------------------------------------------- kernel build
def build_nc(t_steps=T):
    nc = bass.Bass()
    # xt is padded by one chunk so the pipelined gx prefetch can run one
    # chunk ahead without a guard.
    xt_ext = nc.declare_dram_parameter(
        "xt", [D, (t_steps + CH) * BL], F16, isOutput=False
    )
    wih_ext = nc.declare_dram_parameter("wih", [D, G4], F16, isOutput=False)
    whh_ext = nc.declare_dram_parameter("whh", [H, G4], F16, isOutput=False)
    bias_ext = nc.declare_dram_parameter("bias", [128, M], F32, isOutput=False)
    out_ext = nc.declare_dram_parameter("out", [t_steps * H, BL], F16, isOutput=True)
    cn_ext = nc.declare_dram_parameter("cn", [H, BL], F32, isOutput=True)

    with tile.TileContext(nc) as tc:
        with (
            tc.tile_pool(name="wpool", bufs=1) as wpool,
            tc.tile_pool(name="state", bufs=1) as state,
            tc.tile_pool(name="xstage", bufs=3) as xstage_p,
            tc.tile_pool(name="hstage", bufs=2) as hstage_p,
            tc.tile_pool(name="tmp", bufs=6) as tmp_p,
            tc.tile_pool(name="gates", bufs=2) as gates_p,
            tc.tile_pool(name="recps", bufs=1, space="PSUM") as recps_p,
            tc.tile_pool(name="gxps", bufs=2, space="PSUM") as gxps_p,
        ):
            wih_sb = wpool.tile([128, KD * G4], F16, tag="wih")
            whh_sb = wpool.tile([128, KH * G4], F16, tag="whh")
            bias_sb = wpool.tile([128, M], F32, tag="bias")
            for k in range(KD):
                nc.sync.dma_start(
                    wih_sb[:, G4 * k : G4 * (k + 1)],
                    wih_ext[128 * k : 128 * (k + 1), :],
                )
            for k in range(KH):
                nc.sync.dma_start(
                    whh_sb[:, G4 * k : G4 * (k + 1)],
                    whh_ext[128 * k : 128 * (k + 1), :],
                )
            nc.sync.dma_start(bias_sb[:], bias_ext[:])

            h16 = state.tile([128, KH * BL], F16, tag="h16")  # h^T, chunk k at 16k
            c32 = state.tile([128, KH * BL], F32, tag="c32")  # c^T
            gx_a = state.tile([128, M * CW], F16, tag="gx_a")
            gx_b = state.tile([128, M * CW], F16, tag="gx_b")
            nc.vector.memset(h16[:], 0.0)
            nc.vector.memset(c32[:], 0.0)

            def gx_phase(xcol, gxc):
                """gxc[:, m*CW + t*BL + b] = (w_ih @ x^T + bias) for one chunk."""
                xst = xstage_p.tile([128, KD * CW], F16, tag="xst")
                for k in range(KD):
                    nc.sync.dma_start(
                        xst[:, CW * k : CW * (k + 1)],
                        xt_ext[128 * k : 128 * (k + 1), bass.ds(xcol, CW)],
                    )
                for m in range(M):
                    gps = gxps_p.tile([128, CW], F32, tag="gps")
                    for k in range(KD):
                        nc.tensor.matmul(
                            gps[:],
                            wih_sb[:, G4 * k + 128 * m : G4 * k + 128 * (m + 1)],
                            xst[:, CW * k : CW * (k + 1)],
                            start=(k == 0),
                            stop=(k == KD - 1),
                        )
                    nc.vector.tensor_scalar_add(
                        gxc[:, CW * m : CW * (m + 1)], gps[:], bias_sb[:, m : m + 1]
                    )

            def rec_phase(trow, gxc, xcol_next, gx_next):
                """Run CH recurrence steps consuming gxc; write h chunk out.

                h lives in fp16 directly (hst slices double as the next
                step's matmul operand); h16 carries state across chunks.
                The NEXT chunk's gx matmuls are interleaved 4 gate-tiles
                per step so they fill the PE stalls left by each step's
                serial epilogue.
                """
                gxv = gxc.rearrange("p (m c) -> p m c", m=M)
                hst = hstage_p.tile([128, CH * KH * BL], F16, tag="hst")
                xst = xstage_p.tile([128, KD * CW], F16, tag="xst")
                for k in range(KD):
                    nc.sync.dma_start(
                        xst[:, CW * k : CW * (k + 1)],
                        xt_ext[128 * k : 128 * (k + 1), bass.ds(xcol_next, CW)],
                    )
                h_cur = h16[:]
                for tl in range(CH):
                    rps = [
                        recps_p.tile([128, KH * BL], F32, tag=f"rps{g}", name=f"rps{g}")
                        for g in range(4)
                    ]
                    for m in range(M):
                        g4, mj = divmod(m, KH)
                        for k in range(KH):
                            nc.tensor.matmul(
                                rps[g4][:, BL * mj : BL * (mj + 1)],
                                whh_sb[:, G4 * k + 128 * m : G4 * k + 128 * (m + 1)],
                                h_cur[:, BL * k : BL * (k + 1)],
                                start=(k == 0),
                                stop=(k == KH - 1),
                            )
                    for mj in range(M // CH):
                        mg = (M // CH) * tl + mj
                        gps = gxps_p.tile([128, CW], F32, tag="gps")
                        for k in range(KD):
                            nc.tensor.matmul(
                                gps[:],
                                wih_sb[:, G4 * k + 128 * mg : G4 * k + 128 * (mg + 1)],
                                xst[:, CW * k : CW * (k + 1)],
                                start=(k == 0),
                                stop=(k == KD - 1),
                            )
                        nc.vector.tensor_scalar_add(
                            gx_next[:, CW * mg : CW * (mg + 1)],
                            gps[:],
                            bias_sb[:, mg : mg + 1],
                        )
                    acts = []
                    for g4, func in enumerate(
                        [AF.Sigmoid, AF.Sigmoid, AF.Tanh, AF.Sigmoid]
                    ):
                        tmp = tmp_p.tile([128, KH * BL], F32, tag="pregate")
                        nc.vector.tensor_tensor(
                            tmp.rearrange("p (m c) -> p m c", m=KH),
                            rps[g4].rearrange("p (m c) -> p m c", m=KH),
                            gxv[:, KH * g4 : KH * (g4 + 1), bass.ds(BL * tl, BL)],
                            mybir.AluOpType.add,
                        )
                        gate = gates_p.tile([128, KH * BL], F32, tag=f"gate{g4}")
                        nc.scalar.activation(gate[:], tmp[:], func)
                        acts.append(gate)
                    sig_i, sig_f, tg, sig_o = acts
                    t1 = tmp_p.tile([128, KH * BL], F32, tag="t1")
                    nc.vector.tensor_tensor(
                        t1[:], sig_i[:], tg[:], mybir.AluOpType.mult
                    )
                    nc.vector.tensor_tensor(
                        c32[:], sig_f[:], c32[:], mybir.AluOpType.mult
                    )
                    nc.vector.tensor_tensor(
                        c32[:], c32[:], t1[:], mybir.AluOpType.add
                    )
                    th = tmp_p.tile([128, KH * BL], F32, tag="th")
                    nc.scalar.activation(th[:], c32[:], AF.Tanh)
                    hs = hst[:, KH * BL * tl : KH * BL * (tl + 1)]
                    nc.vector.tensor_tensor(
                        hs, sig_o[:], th[:], mybir.AluOpType.mult
                    )
                    h_cur = hs
                nc.vector.tensor_copy(h16[:], h_cur)
                nc.sync.dma_start(
                    out_ext[bass.ds(trow, CH * H), :].rearrange(
                        "(t k p) b -> p t k b", t=CH, k=KH, p=128
                    ),
                    hst.rearrange("p (t k b) -> p t k b", t=CH, k=KH),
                )

            # prologue: gx for chunk 0
            gx_phase(0, gx_a)
            # chunk pairs: rec(2j) from A computes gx(2j+1)->B inline,
            # then rec(2j+1) from B computes gx(2j+2)->A inline.
            with tc.For_i(0, t_steps * BL, 2 * CW) as xcol:
                rec_phase(
                    nc.snap(xcol * (H // BL)), gx_a, nc.snap(xcol + CW), gx_b
                )
                rec_phase(
                    nc.snap(xcol * (H // BL) + CH * H),
                    gx_b,
                    nc.snap(xcol + 2 * CW),
                    gx_a,
                )

            nc.sync.dma_start(
                cn_ext.rearrange("(k p) b -> p k b", p=128),
                c32.rearrange("p (k b) -> p k b", k=KH),
            )

    _legalize_waits(nc)
    return nc


_NC_CACHE = {}


def _get_nc(t_steps):
    if t_steps not in _NC_CACHE:
        _NC_CACHE[t_steps] = build_nc(t_steps)
    return _NC_CACHE[t_steps]


# ---------------------------------------------------------------- host wrapper
def kernel(x, w_ih, b_ih, w_hh, b_hh, trace=False):
    x = np.asarray(x)
    w_ih = np.asarray(w_ih)
    b_ih = np.asarray(b_ih)
    w_hh = np.asarray(w_hh)
    b_hh = np.asarray(b_hh)
    t_steps = x.shape[0]

    _install_ntff_hook()
    nc = _get_nc(t_steps)

    wihT = np.ascontiguousarray(w_ih.T).astype(np.float16)  # [D, 4H]
    whhT = np.ascontiguousarray(w_hh.T).astype(np.float16)  # [H, 4H]
    bias = (b_ih + b_hh).astype(np.float32).reshape(M, 128).T.copy()  # [128, M]

    in_maps = []
    for c in range(NCORES):
        sl = slice(BL * c, BL * (c + 1))
        # x^T slice: [D, T, BL] -> [D, (T+CH)*BL] fp16, padded one chunk
        xt = np.zeros((D, (t_steps + CH) * BL), np.float16)
        xt[:, : t_steps * BL] = (
            np.ascontiguousarray(x[:, sl, :].transpose(2, 0, 1))
            .reshape(D, t_steps * BL)
            .astype(np.float16)
        )
        in_maps.append(
            {"xt": xt, "wih": wihT.copy(), "whh": whhT.copy(), "bias": bias.copy()}
        )

    res = run_bass_kernel_spmd(nc, in_maps, list(range(NCORES)), trace=trace)

    out = np.empty((t_steps, B, H), np.float32)
    c_n = np.empty((B, H), np.float32)
    for c in range(NCORES):
        sl = slice(BL * c, BL * (c + 1))
        oc = res.results[c]["out"].reshape(t_steps, H, BL).astype(np.float32)
        out[:, sl, :] = oc.transpose(0, 2, 1)
        c_n[sl, :] = res.results[c]["cn"].T
    h_n = out[-1].copy()
    if trace:
        kernel.last_exec_ns = res.exec_time_ns
    return out, (h_n, c_n)


# revision 24
# speedup vs baseline: 1.3306x; 1.0810x over previous
"""Trainium2 Bass kernel for single-layer LSTM (DPLSTM forward).

Problem: x [T=512, B=128, D=1024], w_ih [4096, 1024], w_hh [4096, 1024],
b_ih/b_hh [4096]. Returns (out [T,B,H], (h_n [B,H], c_n [B,H])).

Strategy (batch-parallel across 8 NeuronCores, per the sharding hint):
  - each core owns a 16-element batch slice; weights replicated.
  - transposed layout on-chip: gates^T [4096, 16] (gate rows on SBUF
    partitions, batch on the free dim) so all elementwise work runs on
    full 128-lane tiles and h^T feeds the next matmul without transposes.
  - fp16 matmuls (fp32 matmul is 4 cycles/row on TRN2; fp16 is 1),
    fp32 PSUM accumulation and fp32 cell state.
  - input projections gx = w_ih @ x^T + bias are computed one 8-step
    chunk AHEAD of the recurrence (software pipeline with two buffers),
    so the gx matmuls fill the PE stalls left by the per-step serial
    epilogue (activations + cell update).
  - per-gate PSUM tiles (4 banks) let gate i's epilogue overlap gate
    f/g/o matmuls.
  - one For_i loop over 32 chunk-pairs keeps the program small.

Host side shards/transposes inputs per core (numpy), runs the SPMD
kernel via run_bass_kernel_spmd on cores 0-7, and reassembles the full
[T, B, H] output (device writes [T, H, 16] per core; final transpose is
host-side numpy).
"""

import contextlib
import ctypes
import sys
import types

import numpy as np

import bass_rust
import concourse.bass as bass
import concourse.mybir as mybir
import concourse.tile as tile
from concourse.bass_utils import run_bass_kernel_spmd

T, B, D, H = 512, 128, 1024, 1024
G4 = 4 * H  # 4096 gate rows
NCORES = 8
BL = B // NCORES  # 16 batch per core
CH = 8  # timesteps per chunk
KD = D // 128  # 8 k-chunks over D
KH = H // 128  # 8 k-chunks over H
M = G4 // 128  # 32 gate partition-tiles
CW = CH * BL  # moving columns per chunk (128)

F16 = mybir.dt.float16
F32 = mybir.dt.float32
AF = mybir.ActivationFunctionType


# ---------------------------------------------------------------- infra
def _legalize_waits(nc):
    """This walrus accepts at most ONE sem-wait per instruction; hoist
    extras onto InstEventSemaphore instructions inserted just before."""
    cnt = 0
    for fn in nc.m.functions:
        for bb in fn.blocks:
            new_list = []
            changed = False
            for inst in bb.instructions:
                si = inst.sync_info
                if si is not None and len(si.on_wait) > 1:
                    waits = list(si.on_wait)
                    for w in waits[:-1]:
                        cnt += 1
                        ev = mybir.InstEventSemaphore(
                            name=f"I-lw-{cnt}", ins=[], outs=[]
                        )
                        ev.engine = inst.engine
                        ev.sync_info = bass_rust.SyncInfo(on_wait=[w], on_update=[])
                        new_list.append(ev)
                    inst.sync_info = bass_rust.SyncInfo(
                        on_wait=[waits[-1]], on_update=list(si.on_update)
                    )
                    changed = True
                new_list.append(inst)
            if changed:
                bb.instructions = new_list


def _install_ntff_hook(so_path="/opt/axon/libaxon_pjrt.so"):
    if "antenv.axon_hooks" in sys.modules:
        return
    try:
        lib = ctypes.CDLL(so_path)
    except OSError:
        return
    if not hasattr(lib, "axon_start_nrt_profile"):
        return
    lib.axon_start_nrt_profile.argtypes = [
        ctypes.POINTER(ctypes.c_int64),
        ctypes.c_size_t,
    ]
    lib.axon_start_nrt_profile.restype = ctypes.c_int64
    lib.axon_stop_nrt_profile.argtypes = [ctypes.c_char_p]
    lib.axon_stop_nrt_profile.restype = ctypes.c_int64

    @contextlib.contextmanager
    def _hook(output_dir, device_ids):
        import jax

        jax.devices()
        if device_ids:
            ids = (ctypes.c_int64 * len(device_ids))(*device_ids)
            rc = lib.axon_start_nrt_profile(ids, len(device_ids))
        else:
            rc = lib.axon_start_nrt_profile(None, 0)
        if rc != 0:
            raise RuntimeError(f"axon_start_nrt_profile rc={rc}")
        try:
            yield
        finally:
            n = lib.axon_stop_nrt_profile(str(output_dir).encode())
            print(f"profile: {n} ntff file(s) -> {output_dir}", file=sys.stderr)

    mod = types.ModuleType("antenv.axon_hooks")
    mod.get_axon_ntff_profile_hook = lambda: _hook
    mod.set_axon_ntff_profile_hook = lambda h: None
    sys.modules["antenv.axon_hooks"] = mod


# ---------------------------------------------------------------- kernel build
def build_nc(t_steps=T):
    nc = bass.Bass()
    # xt is padded by one chunk so the pipelined gx prefetch can run one
    # chunk ahead without a guard.
    xt_ext = nc.declare_dram_parameter(
        "xt", [D, (t_steps + CH) * BL], F16, isOutput=False
    )
    wih_ext = nc.declare_dram_parameter("wih", [D, G4], F16, isOutput=False)
    whh_ext = nc.declare_dram_parameter("whh", [H, G4], F16, isOutput=False)
    bias_ext = nc.declare_dram_parameter("bias", [128, M], F32, isOutput=False)
    out_ext = nc.declare_dram_parameter("out", [t_steps * H, BL], F16, isOutput=True)
    cn_ext = nc.declare_dram_parameter("cn", [H, BL], F32, isOutput=True)

    with tile.TileContext(nc) as tc:
        with (
            tc.tile_pool(name="wpool", bufs=1) as wpool,
            tc.tile_pool(name="state", bufs=1) as state,
            tc.tile_pool(name="xstage", bufs=3) as xstage_p,
            tc.tile_pool(name="hstage", bufs=2) as hstage_p,
            tc.tile_pool(name="tmp", bufs=6) as tmp_p,
            tc.tile_pool(name="gates", bufs=2) as gates_p,
            tc.tile_pool(name="recps", bufs=1, space="PSUM") as recps_p,
            tc.tile_pool(name="gxps", bufs=2, space="PSUM") as gxps_p,
        ):
            wih_sb = wpool.tile([128, KD * G4], F16, tag="wih")
            whh_sb = wpool.tile([128, KH * G4], F16, tag="whh")
            bias_sb = wpool.tile([128, M], F32, tag="bias")
            for k in range(KD):
                nc.sync.dma_start(
                    wih_sb[:, G4 * k : G4 * (k + 1)],
                    wih_ext[128 * k : 128 * (k + 1), :],
                )
            for k in range(KH):
                nc.sync.dma_start(
                    whh_sb[:, G4 * k : G4 * (k + 1)],
                    whh_ext[128 * k : 128 * (k + 1), :],
                )
            nc.sync.dma_start(bias_sb[:], bias_ext[:])

            # h^T state, chunk k at 16k; padded 64 cols so the widened
            # (N=64) matmul moving reads stay in-bounds (see rec_phase).
            h16 = state.tile([128, KH * BL + 48], F16, tag="h16")
            c32 = state.tile([128, KH * BL], F32, tag="c32")  # c^T
            gx_a = state.tile([128, M * CW], F16, tag="gx_a")
            gx_b = state.tile([128, M * CW], F16, tag="gx_b")
            nc.vector.memset(h16[:], 0.0)
            nc.vector.memset(c32[:], 0.0)

            def gx_phase(xcol, gxc):
                """gxc[:, m*CW + t*BL + b] = (w_ih @ x^T + bias) for one chunk."""
                xst = xstage_p.tile([128, KD * CW], F16, tag="xst")
                for k in range(KD):
                    nc.sync.dma_start(
                        xst[:, CW * k : CW * (k + 1)],
                        xt_ext[128 * k : 128 * (k + 1), bass.ds(xcol, CW)],
                    )
                for m in range(M):
                    gps = gxps_p.tile([128, CW], F32, tag="gps")
                    for k in range(KD):
                        nc.tensor.matmul(
                            gps[:],
                            wih_sb[:, G4 * k + 128 * m : G4 * k + 128 * (m + 1)],
                            xst[:, CW * k : CW * (k + 1)],
                            start=(k == 0),
                            stop=(k == KD - 1),
                        )
                    nc.vector.tensor_scalar_add(
                        gxc[:, CW * m : CW * (m + 1)], gps[:], bias_sb[:, m : m + 1]
                    )

            def rec_phase(trow, gxc, xcol_next, gx_next):
                """Run CH recurrence steps consuming gxc; write h chunk out.

                h lives in fp16 directly (hst slices double as the next
                step's matmul operand); h16 carries state across chunks.
                The NEXT chunk's gx matmuls are interleaved 4 gate-tiles
                per step so they fill the PE stalls left by each step's
                serial epilogue.

                The recurrence matmuls stream N=64 moving columns (16
                real batch + 48 don't-care) purely to keep the PE array's
                duty cycle high enough that the HAM clock gate holds
                K=8/8 (2.4 GHz): at N=16 the trace shows the PE throttled
                to 1.2 GHz for most of each chunk. The extra columns cost
                nothing (the pair is weight-load-bound either way) and
                land in PSUM columns nothing reads.
                """
                NW = 64  # widened moving/psum column group
                gxv = gxc.rearrange("p (m c) -> p m c", m=M)
                hst = hstage_p.tile([128, CH * KH * BL + 48], F16, tag="hst")
                xst = xstage_p.tile([128, KD * CW], F16, tag="xst")
                for k in range(KD):
                    nc.sync.dma_start(
                        xst[:, CW * k : CW * (k + 1)],
                        xt_ext[128 * k : 128 * (k + 1), bass.ds(xcol_next, CW)],
                    )
                h_cur = h16[:]
                for tl in range(CH):
                    rps = [
                        recps_p.tile([128, KH * NW], F32, tag=f"rps{g}", name=f"rps{g}")
                        for g in range(4)
                    ]
                    for m in range(M):
                        g4, mj = divmod(m, KH)
                        for k in range(KH):
                            nc.tensor.matmul(
                                rps[g4][:, NW * mj : NW * (mj + 1)],
                                whh_sb[:, G4 * k + 128 * m : G4 * k + 128 * (m + 1)],
                                h_cur[:, BL * k : BL * k + NW],
                                start=(k == 0),
                                stop=(k == KH - 1),
                            )
                    for mj in range(M // CH):
                        mg = (M // CH) * tl + mj
                        gps = gxps_p.tile([128, CW], F32, tag="gps")
                        for k in range(KD):
                            nc.tensor.matmul(
                                gps[:],
                                wih_sb[:, G4 * k + 128 * mg : G4 * k + 128 * (mg + 1)],
                                xst[:, CW * k : CW * (k + 1)],
                                start=(k == 0),
                                stop=(k == KD - 1),
                            )
                        nc.vector.tensor_scalar_add(
                            gx_next[:, CW * mg : CW * (mg + 1)],
                            gps[:],
                            bias_sb[:, mg : mg + 1],
                        )
                    acts = []
                    for g4, func in enumerate(
                        [AF.Sigmoid, AF.Sigmoid, AF.Tanh, AF.Sigmoid]
                    ):
                        tmp = tmp_p.tile([128, KH * BL], F32, tag="pregate")
                        nc.vector.tensor_tensor(
                            tmp.rearrange("p (m c) -> p m c", m=KH),
                            rps[g4].rearrange("p (m c) -> p m c", m=KH)[:, :, 0:BL],
                            gxv[:, KH * g4 : KH * (g4 + 1), bass.ds(BL * tl, BL)],
                            mybir.AluOpType.add,
                        )
                        gate = gates_p.tile([128, KH * BL], F32, tag=f"gate{g4}")
                        nc.scalar.activation(gate[:], tmp[:], func)
                        acts.append(gate)
                    sig_i, sig_f, tg, sig_o = acts
                    t1 = tmp_p.tile([128, KH * BL], F32, tag="t1")
                    nc.vector.tensor_tensor(
                        t1[:], sig_i[:], tg[:], mybir.AluOpType.mult
                    )
                    nc.vector.tensor_tensor(
                        c32[:], sig_f[:], c32[:], mybir.AluOpType.mult
                    )
                    nc.vector.tensor_tensor(
                        c32[:], c32[:], t1[:], mybir.AluOpType.add
                    )
                    th = tmp_p.tile([128, KH * BL], F32, tag="th")
                    nc.scalar.activation(th[:], c32[:], AF.Tanh)
                    hs = hst[:, KH * BL * tl : KH * BL * (tl + 1)]
                    nc.vector.tensor_tensor(
                        hs, sig_o[:], th[:], mybir.AluOpType.mult
                    )
                    # read view widened by 48 cols for the N=64 moving reads
                    h_cur = hst[:, KH * BL * tl : KH * BL * (tl + 1) + 48]
                nc.vector.tensor_copy(h16[:, : KH * BL], h_cur[:, : KH * BL])
                nc.sync.dma_start(
                    out_ext[bass.ds(trow, CH * H), :].rearrange(
                        "(t k p) b -> p t k b", t=CH, k=KH, p=128
                    ),
                    hst[:, : CH * KH * BL].rearrange(
                        "p (t k b) -> p t k b", t=CH, k=KH
                    ),
                )

            # prologue: gx for chunk 0
            gx_phase(0, gx_a)
            # chunk pairs: rec(2j) from A computes gx(2j+1)->B inline,
            # then rec(2j+1) from B computes gx(2j+2)->A inline.
            with tc.For_i(0, t_steps * BL, 2 * CW) as xcol:
                rec_phase(
                    nc.snap(xcol * (H // BL)), gx_a, nc.snap(xcol + CW), gx_b
                )
                rec_phase(
                    nc.snap(xcol * (H // BL) + CH * H),
                    gx_b,
                    nc.snap(xcol + 2 * CW),
                    gx_a,
                )

            nc.sync.dma_start(
                cn_ext.rearrange("(k p) b -> p k b", p=128),
                c32.rearrange("p (k b) -> p k b", k=KH),
            )

    _legalize_waits(nc)
    return nc


_NC_CACHE = {}


def _get_nc(t_steps):
    if t_steps not in _NC_CACHE:
        _NC_CACHE[t_steps] = build_nc(t_steps)
    return _NC_CACHE[t_steps]


# ---------------------------------------------------------------- host wrapper
def kernel(x, w_ih, b_ih, w_hh, b_hh, trace=False):
    x = np.asarray(x)
    w_ih = np.asarray(w_ih)
    b_ih = np.asarray(b_ih)
    w_hh = np.asarray(w_hh)
    b_hh = np.asarray(b_hh)
    t_steps = x.shape[0]

    _install_ntff_hook()
    nc = _get_nc(t_steps)

    wihT = np.ascontiguousarray(w_ih.T).astype(np.float16)  # [D, 4H]
    whhT = np.ascontiguousarray(w_hh.T).astype(np.float16)  # [H, 4H]
    bias = (b_ih + b_hh).astype(np.float32).reshape(M, 128).T.copy()  # [128, M]

    in_maps = []
    for c in range(NCORES):
        sl = slice(BL * c, BL * (c + 1))
        # x^T slice: [D, T, BL] -> [D, (T+CH)*BL] fp16, padded one chunk
        xt = np.zeros((D, (t_steps + CH) * BL), np.float16)
        xt[:, : t_steps * BL] = (
            np.ascontiguousarray(x[:, sl, :].transpose(2, 0, 1))
            .reshape(D, t_steps * BL)
            .astype(np.float16)
        )
        in_maps.append(
            {"xt": xt, "wih": wihT.copy(), "whh": whhT.copy(), "bias": bias.copy()}
        )

    res = run_bass_kernel_spmd(nc, in_maps, list(range(NCORES)), trace=trace)

    out = np.empty((t_steps, B, H), np.float32)
    c_n = np.empty((B, H), np.float32)
    for c in range(NCORES):
        sl = slice(BL * c, BL * (c + 1))
        oc = res.results[c]["out"].reshape(t_steps, H, BL).astype(np.float32)
        out[:, sl, :] = oc.transpose(0, 2, 1)
        c_n[sl, :] = res.results[c]["cn"].T
    h_n = out[-1].copy()
    if trace:
        kernel.last_exec_ns = res.exec_time_ns
    return out, (h_n, c_n)


# revision 28
# speedup vs baseline: 1.4000x; 1.0521x over previous
"""Trainium2 Bass kernel for single-layer LSTM (DPLSTM forward).

Problem: x [T=512, B=128, D=1024], w_ih [4096, 1024], w_hh [4096, 1024],
b_ih/b_hh [4096]. Returns (out [T,B,H], (h_n [B,H], c_n [B,H])).

Strategy (batch-parallel across 8 NeuronCores, per the sharding hint):
  - each core owns a 16-element batch slice; weights replicated.
  - transposed layout on-chip: gates^T [4096, 16] (gate rows on SBUF
    partitions, batch on the free dim) so all elementwise work runs on
    full 128-lane tiles and h^T feeds the next matmul without transposes.
  - fp16 matmuls (fp32 matmul is 4 cycles/row on TRN2; fp16 is 1),
    fp32 PSUM accumulation and fp32 cell state.
  - input projections gx = w_ih @ x^T + bias are computed one 8-step
    chunk AHEAD of the recurrence (software pipeline with two buffers),
    so the gx matmuls fill the PE stalls left by the per-step serial
    epilogue (activations + cell update).
  - per-gate PSUM tiles (4 banks) let gate i's epilogue overlap gate
    f/g/o matmuls.
  - one For_i loop over 32 chunk-pairs keeps the program small.

Host side shards/transposes inputs per core (numpy), runs the SPMD
kernel via run_bass_kernel_spmd on cores 0-7, and reassembles the full
[T, B, H] output (device writes [T, H, 16] per core; final transpose is
host-side numpy).
"""

import contextlib
import ctypes
import sys
import types

import numpy as np

import bass_rust
import concourse.bass as bass
import concourse.mybir as mybir
import concourse.tile as tile
from concourse.bass_utils import run_bass_kernel_spmd

T, B, D, H = 512, 128, 1024, 1024
G4 = 4 * H  # 4096 gate rows
NCORES = 8
BL = B // NCORES  # 16 batch per core
CH = 8  # timesteps per chunk
KD = D // 128  # 8 k-chunks over D
KH = H // 128  # 8 k-chunks over H
M = G4 // 128  # 32 gate partition-tiles
CW = CH * BL  # moving columns per chunk (128)

F16 = mybir.dt.float16
F32 = mybir.dt.float32
AF = mybir.ActivationFunctionType


# ---------------------------------------------------------------- infra
def _legalize_waits(nc):
    """This walrus accepts at most ONE sem-wait per instruction; hoist
    extras onto InstEventSemaphore instructions inserted just before."""
    cnt = 0
    for fn in nc.m.functions:
        for bb in fn.blocks:
            new_list = []
            changed = False
            for inst in bb.instructions:
                si = inst.sync_info
                if si is not None and len(si.on_wait) > 1:
                    waits = list(si.on_wait)
                    for w in waits[:-1]:
                        cnt += 1
                        ev = mybir.InstEventSemaphore(
                            name=f"I-lw-{cnt}", ins=[], outs=[]
                        )
                        ev.engine = inst.engine
                        ev.sync_info = bass_rust.SyncInfo(on_wait=[w], on_update=[])
                        new_list.append(ev)
                    inst.sync_info = bass_rust.SyncInfo(
                        on_wait=[waits[-1]], on_update=list(si.on_update)
                    )
                    changed = True
                new_list.append(inst)
            if changed:
                bb.instructions = new_list


def _install_ntff_hook(so_path="/opt/axon/libaxon_pjrt.so"):
    if "antenv.axon_hooks" in sys.modules:
        return
    try:
        lib = ctypes.CDLL(so_path)
    except OSError:
        return
    if not hasattr(lib, "axon_start_nrt_profile"):
        return
    lib.axon_start_nrt_profile.argtypes = [
        ctypes.POINTER(ctypes.c_int64),
        ctypes.c_size_t,
    ]
    lib.axon_start_nrt_profile.restype = ctypes.c_int64
    lib.axon_stop_nrt_profile.argtypes = [ctypes.c_char_p]
    lib.axon_stop_nrt_profile.restype = ctypes.c_int64

    @contextlib.contextmanager
    def _hook(output_dir, device_ids):
        import jax

        jax.devices()
        if device_ids:
            ids = (ctypes.c_int64 * len(device_ids))(*device_ids)
            rc = lib.axon_start_nrt_profile(ids, len(device_ids))
        else:
            rc = lib.axon_start_nrt_profile(None, 0)
        if rc != 0:
            raise RuntimeError(f"axon_start_nrt_profile rc={rc}")
        try:
            yield
        finally:
            n = lib.axon_stop_nrt_profile(str(output_dir).encode())
            print(f"profile: {n} ntff file(s) -> {output_dir}", file=sys.stderr)

    mod = types.ModuleType("antenv.axon_hooks")
    mod.get_axon_ntff_profile_hook = lambda: _hook
    mod.set_axon_ntff_profile_hook = lambda h: None
    sys.modules["antenv.axon_hooks"] = mod


# ---------------------------------------------------------------- kernel build
def build_nc(t_steps=T):
    nc = bass.Bass()
    # xt is padded by one chunk so the pipelined gx prefetch can run one
    # chunk ahead without a guard.
    xt_ext = nc.declare_dram_parameter(
        "xt", [D, (t_steps + CH) * BL], F16, isOutput=False
    )
    wih_ext = nc.declare_dram_parameter("wih", [D, G4], F16, isOutput=False)
    whh_ext = nc.declare_dram_parameter("whh", [H, G4], F16, isOutput=False)
    bias_ext = nc.declare_dram_parameter("bias", [128, M], F32, isOutput=False)
    # partition-major output: out[p, (t, k, b)] = h[t, 128k+p, b]; each
    # partition's run is contiguous in DRAM (fast DMA), host untangles it.
    out_ext = nc.declare_dram_parameter(
        "out", [128, t_steps * KH * BL], F16, isOutput=True
    )
    cn_ext = nc.declare_dram_parameter("cn", [H, BL], F32, isOutput=True)

    with tile.TileContext(nc) as tc:
        with (
            tc.tile_pool(name="wpool", bufs=1) as wpool,
            tc.tile_pool(name="state", bufs=1) as state,
            tc.tile_pool(name="xstage", bufs=3) as xstage_p,
            tc.tile_pool(name="hstage", bufs=2) as hstage_p,
            tc.tile_pool(name="tmp", bufs=6) as tmp_p,
            tc.tile_pool(name="gates", bufs=2) as gates_p,
            tc.tile_pool(name="recps", bufs=1, space="PSUM") as recps_p,
            tc.tile_pool(name="gxps", bufs=2, space="PSUM") as gxps_p,
        ):
            wih_sb = wpool.tile([128, KD * G4], F16, tag="wih")
            whh_sb = wpool.tile([128, KH * G4], F16, tag="whh")
            bias_sb = wpool.tile([128, M], F32, tag="bias")
            for k in range(KD):
                nc.sync.dma_start(
                    wih_sb[:, G4 * k : G4 * (k + 1)],
                    wih_ext[128 * k : 128 * (k + 1), :],
                )
            for k in range(KH):
                nc.sync.dma_start(
                    whh_sb[:, G4 * k : G4 * (k + 1)],
                    whh_ext[128 * k : 128 * (k + 1), :],
                )
            nc.sync.dma_start(bias_sb[:], bias_ext[:])

            # h^T state, chunk k at 16k; padded 64 cols so the widened
            # (N=64) matmul moving reads stay in-bounds (see rec_phase).
            h16 = state.tile([128, KH * BL + 48], F16, tag="h16")
            c32 = state.tile([128, KH * BL], F32, tag="c32")  # c^T
            gx_a = state.tile([128, M * CW], F16, tag="gx_a")
            gx_b = state.tile([128, M * CW], F16, tag="gx_b")
            nc.vector.memset(h16[:], 0.0)
            nc.vector.memset(c32[:], 0.0)

            def gx_phase(xcol, gxc):
                """gxc[:, m*CW + t*BL + b] = (w_ih @ x^T + bias) for one chunk."""
                xst = xstage_p.tile([128, KD * CW], F16, tag="xst")
                for k in range(KD):
                    nc.sync.dma_start(
                        xst[:, CW * k : CW * (k + 1)],
                        xt_ext[128 * k : 128 * (k + 1), bass.ds(xcol, CW)],
                    )
                for m in range(M):
                    gps = gxps_p.tile([128, CW], F32, tag="gps")
                    for k in range(KD):
                        nc.tensor.matmul(
                            gps[:],
                            wih_sb[:, G4 * k + 128 * m : G4 * k + 128 * (m + 1)],
                            xst[:, CW * k : CW * (k + 1)],
                            start=(k == 0),
                            stop=(k == KD - 1),
                        )
                    nc.vector.tensor_scalar_add(
                        gxc[:, CW * m : CW * (m + 1)], gps[:], bias_sb[:, m : m + 1]
                    )

            def rec_phase(trow, gxc, xcol_next, gx_next):
                """Run CH recurrence steps consuming gxc; write h chunk out.

                h lives in fp16 directly (hst slices double as the next
                step's matmul operand); h16 carries state across chunks.
                The NEXT chunk's gx matmuls are interleaved 4 gate-tiles
                per step so they fill the PE stalls left by each step's
                serial epilogue.

                The recurrence matmuls stream N=64 moving columns (16
                real batch + 48 don't-care) purely to keep the PE array's
                duty cycle high enough that the HAM clock gate holds
                K=8/8 (2.4 GHz): at N=16 the trace shows the PE throttled
                to 1.2 GHz for most of each chunk. The extra columns cost
                nothing (the pair is weight-load-bound either way) and
                land in PSUM columns nothing reads.
                """
                NW = 64  # widened moving/psum column group
                gxv = gxc.rearrange("p (m c) -> p m c", m=M)
                hst = hstage_p.tile([128, CH * KH * BL + 48], F16, tag="hst")
                xst = xstage_p.tile([128, KD * CW], F16, tag="xst")
                for k in range(KD):
                    nc.sync.dma_start(
                        xst[:, CW * k : CW * (k + 1)],
                        xt_ext[128 * k : 128 * (k + 1), bass.ds(xcol_next, CW)],
                    )
                h_cur = h16[:]
                for tl in range(CH):
                    rps = [
                        recps_p.tile([128, KH * NW], F32, tag=f"rps{g}", name=f"rps{g}")
                        for g in range(4)
                    ]
                    for m in range(M):
                        g4, mj = divmod(m, KH)
                        for k in range(KH):
                            nc.tensor.matmul(
                                rps[g4][:, NW * mj : NW * (mj + 1)],
                                whh_sb[:, G4 * k + 128 * m : G4 * k + 128 * (m + 1)],
                                h_cur[:, BL * k : BL * k + NW],
                                start=(k == 0),
                                stop=(k == KH - 1),
                            )
                    for mj in range(M // CH):
                        mg = (M // CH) * tl + mj
                        gps = gxps_p.tile([128, CW], F32, tag="gps")
                        for k in range(KD):
                            nc.tensor.matmul(
                                gps[:],
                                wih_sb[:, G4 * k + 128 * mg : G4 * k + 128 * (mg + 1)],
                                xst[:, CW * k : CW * (k + 1)],
                                start=(k == 0),
                                stop=(k == KD - 1),
                            )
                        nc.vector.tensor_scalar_add(
                            gx_next[:, CW * mg : CW * (mg + 1)],
                            gps[:],
                            bias_sb[:, mg : mg + 1],
                        )
                    acts = []
                    for g4, func in enumerate(
                        [AF.Sigmoid, AF.Sigmoid, AF.Tanh, AF.Sigmoid]
                    ):
                        tmp = tmp_p.tile([128, KH * BL], F32, tag="pregate")
                        nc.vector.tensor_tensor(
                            tmp.rearrange("p (m c) -> p m c", m=KH),
                            rps[g4].rearrange("p (m c) -> p m c", m=KH)[:, :, 0:BL],
                            gxv[:, KH * g4 : KH * (g4 + 1), bass.ds(BL * tl, BL)],
                            mybir.AluOpType.add,
                        )
                        gate = gates_p.tile([128, KH * BL], F32, tag=f"gate{g4}")
                        nc.scalar.activation(gate[:], tmp[:], func)
                        acts.append(gate)
                    sig_i, sig_f, tg, sig_o = acts
                    t1 = tmp_p.tile([128, KH * BL], F32, tag="t1")
                    nc.vector.tensor_tensor(
                        t1[:], sig_i[:], tg[:], mybir.AluOpType.mult
                    )
                    nc.vector.tensor_tensor(
                        c32[:], sig_f[:], c32[:], mybir.AluOpType.mult
                    )
                    nc.vector.tensor_tensor(
                        c32[:], c32[:], t1[:], mybir.AluOpType.add
                    )
                    th = tmp_p.tile([128, KH * BL], F32, tag="th")
                    nc.scalar.activation(th[:], c32[:], AF.Tanh)
                    hs = hst[:, KH * BL * tl : KH * BL * (tl + 1)]
                    nc.vector.tensor_tensor(
                        hs, sig_o[:], th[:], mybir.AluOpType.mult
                    )
                    # read view widened by 48 cols for the N=64 moving reads
                    h_cur = hst[:, KH * BL * tl : KH * BL * (tl + 1) + 48]
                nc.vector.tensor_copy(h16[:, : KH * BL], h_cur[:, : KH * BL])
                nc.sync.dma_start(
                    out_ext[:, bass.ds(trow, CH * KH * BL)],
                    hst[:, : CH * KH * BL],
                )

            # prologue: gx for chunk 0
            gx_phase(0, gx_a)
            # chunk pairs: rec(2j) from A computes gx(2j+1)->B inline,
            # then rec(2j+1) from B computes gx(2j+2)->A inline.
            with tc.For_i(0, t_steps * BL, 2 * CW) as xcol:
                rec_phase(nc.snap(xcol * KH), gx_a, nc.snap(xcol + CW), gx_b)
                rec_phase(
                    nc.snap(xcol * KH + CH * KH * BL),
                    gx_b,
                    nc.snap(xcol + 2 * CW),
                    gx_a,
                )

            nc.sync.dma_start(
                cn_ext.rearrange("(k p) b -> p k b", p=128),
                c32.rearrange("p (k b) -> p k b", k=KH),
            )

    _legalize_waits(nc)
    return nc


_NC_CACHE = {}


def _get_nc(t_steps):
    if t_steps not in _NC_CACHE:
        _NC_CACHE[t_steps] = build_nc(t_steps)
    return _NC_CACHE[t_steps]


# ---------------------------------------------------------------- host wrapper
def kernel(x, w_ih, b_ih, w_hh, b_hh, trace=False):
    x = np.asarray(x)
    w_ih = np.asarray(w_ih)
    b_ih = np.asarray(b_ih)
    w_hh = np.asarray(w_hh)
    b_hh = np.asarray(b_hh)
    t_steps = x.shape[0]

    _install_ntff_hook()
    nc = _get_nc(t_steps)

    wihT = np.ascontiguousarray(w_ih.T).astype(np.float16)  # [D, 4H]
    whhT = np.ascontiguousarray(w_hh.T).astype(np.float16)  # [H, 4H]
    bias = (b_ih + b_hh).astype(np.float32).reshape(M, 128).T.copy()  # [128, M]

    in_maps = []
    for c in range(NCORES):
        sl = slice(BL * c, BL * (c + 1))
        # x^T slice: [D, T, BL] -> [D, (T+CH)*BL] fp16, padded one chunk
        xt = np.zeros((D, (t_steps + CH) * BL), np.float16)
        xt[:, : t_steps * BL] = (
            np.ascontiguousarray(x[:, sl, :].transpose(2, 0, 1))
            .reshape(D, t_steps * BL)
            .astype(np.float16)
        )
        in_maps.append(
            {"xt": xt, "wih": wihT.copy(), "whh": whhT.copy(), "bias": bias.copy()}
        )

    res = run_bass_kernel_spmd(nc, in_maps, list(range(NCORES)), trace=trace)

    out = np.empty((t_steps, B, H), np.float32)
    c_n = np.empty((B, H), np.float32)
    for c in range(NCORES):
        sl = slice(BL * c, BL * (c + 1))
        # oc[p, t, k, b] -> out[t, b, 128k+p]
        oc = res.results[c]["out"].reshape(128, t_steps, KH, BL).astype(np.float32)
        out[:, sl, :] = oc.transpose(1, 3, 2, 0).reshape(t_steps, BL, H)
        c_n[sl, :] = res.results[c]["cn"].T
    h_n = out[-1].copy()
    if trace:
        kernel.last_exec_ns = res.exec_time_ns
    return out, (h_n, c_n)
